# revision 7
# baseline (speedup 1.0000x reference)
# Trainium2 Bass kernel for nn_AttnModel_64098091926054.
#
# Strategy: pure data parallel over batch (256 boards -> 32 per core x 8 cores).
# Host-side constant folding (softmax shift-invariance kills the x-dependent
# k-term; q_w folds into qk_w (512x19); kvx_v/Wv fold through fin_w).
#
# v5: chain-latency cuts on top of v4:
#  - gconst folded into a per-layer bfpj pad column (bf.gconst precomputed on
#    host) -> no fp32 gconst matmuls; layer 0 skips t1/dots entirely (exp
#    reads the precomputed column with a strided AP).
#  - g3 (128,19) computed directly with step-0-broadcast stationary APs
#    (x columns replicated 4x group-major) -> no g_sb copy, no replicate
#    matmul, no e2tp LDW on the chain.
#  - bfjp carries a ones j-row (JR=20): the t2/s4 reduce emits the softmax
#    row-sum row for free, s4/s4n have even free size (DVE 2x eligible),
#    and sT row 19 lands at exactly 1.0 = the cfin ones row.
#  - fc0/fc1 biases enter PSUM via ONE K=4 matmul each (bias rows x one-hot
#    selector) instead of 4 rank-1s.
#  - head: head_w@posT and fc1_7@(head_w@posT) folded on host; layer-7 fc1
#    never runs on PE; log-softmax skips the max shift (safe in f32); exp
#    reads lg straight from PSUM.

import numpy as np
import ml_dtypes

import concourse.bass as bass
import concourse.bacc as bacc
import concourse.mybir as mybir
import concourse.tile as tile
from concourse.bass_utils import run_bass_kernel_spmd

BS, D, L, B, P, POSD, J = 9, 512, 8, 256, 81, 12, 19
NCORES = 8
NB = B // NCORES          # 32 boards per core
GG, PQ = 4, 21            # 84 = 4 groups x 21 cells (3 pads)
NP = 128                  # partitions: p = gg*32 + b  (group-major)
JP = J + 1                # 20: j padded (pad col = gconst/-40 lane in bfpj)
JR = J + 1                # 20: bfjp j rows (19 features + ones row for rowsum)
PQP = PQ + 1              # 22: pq padded even
NEGBIG = -40.0
OFFSETS = [(-1, 0), (-1, 1), (0, -1), (0, 0), (0, 1), (-1, -1), (-1, 0)]

f32 = mybir.dt.float32
bf16 = mybir.dt.bfloat16
fp8 = mybir.dt.float8e4
bf16_np = ml_dtypes.bfloat16
fp8_np = ml_dtypes.float8_e4m3

# cpk16 columns (bf16): bfjp (20x22) | e2p | e2tp | wz | whz | lgbias | sel4 | b4
C_BFJP = 0
C_E2P = C_BFJP + JR * PQP          # 440
C_E2T = C_E2P + NB                 # 472
C_WZ = C_E2T + NP                  # 600
C_WHZ = C_WZ + 4 * P               # 924
C_LGB = C_WHZ + 4 * P              # 1248
C_SEL = C_LGB + P                  # 1329
C_B4 = C_SEL + NP                  # 1457
C16_END = C_B4 + L * 2 * NP        # 3505
AluOp = mybir.AluOpType
Act = mybir.ActivationFunctionType


def _positions():
    lin = np.linspace(0.0, 1.0, BS, dtype=np.float32)
    rs, cs = np.meshgrid(lin, lin, indexing="ij")
    zs = (rs + cs) / 2.0
    xs = np.stack([rs, cs, zs], -1).astype(np.float32)
    feats = []
    for p in [4.0 / (BS - 1), 16.0 / (BS - 1)]:
        a = (2.0 * np.pi * xs / p).astype(np.float32)
        feats.append(np.concatenate([np.cos(a), np.sin(a)], -1).astype(np.float32))
    return np.concatenate(feats, -1)  # (9, 9, 12)


def _prepare(obs, pos):
    single = obs[..., 0] - obs[..., 1]
    aug = np.pad(single, ((0, 0), (1, 1), (1, 1)))
    w = aug.shape[-1]
    outs = [aug[:, 1 + r : w - 1 + r, 1 + c : w - 1 + c] for (r, c) in OFFSETS]
    neigh = np.stack(outs, -1)
    n = obs.shape[0]
    stack = np.concatenate(
        [neigh, np.broadcast_to(pos, (n,) + pos.shape)], -1
    ).astype(np.float32)
    return stack.reshape(n, P, J)  # (B, 81, 19)


def _fold(inp):
    """Host-side constant folding of weights. All f32 numpy, unscaled."""
    scale = np.float32(1.0 / np.sqrt(D))
    Wk = inp["kvb_w"][:, :, :D]                                   # (L,19,512)
    Wv = inp["kvb_w"][:, :, D:]
    kvx_v = inp["kvx_w"][:, :, D:]                                # (L,512,512)
    qk_w = np.einsum("ldh,ljh->ldj", inp["q_w"], Wk) * scale      # (L,512,19)
    qk_b = np.einsum("lh,ljh->lj", inp["q_b"], Wk) * scale        # (L,19)
    afin = np.einsum("lde,leh->ldh", kvx_v, inp["fin_w"])         # (L,512,512)
    sfin = np.einsum("lje,leh->ljh", Wv, inp["fin_w"])            # (L,19,512)
    bias_v = inp["kvx_b"][:, D:] + inp["kvb_b"][:, D:]
    cfin = np.einsum("le,leh->lh", bias_v, inp["fin_w"]) + inp["fin_b"]
    return qk_w, qk_b, afin, sfin, cfin


def _ktile_lhsT(W):
    """(L,512,512) -> (L,128,2048) with col ((o*4+kt)*128+m) = W[l,kt*128+k,o*128+m]."""
    Lx = W.shape[0]
    return np.ascontiguousarray(
        W.reshape(Lx, 4, 128, 4, 128).transpose(0, 2, 3, 1, 4).reshape(Lx, 128, 2048)
    )


def _build_nc(alpha):
    nc = bacc.Bacc("TRN2", target_bir_lowering=False, debug=False)

    d_cpk16 = nc.dram_tensor("cpk16", [128, C16_END], bf16, kind="ExternalInput")
    d_bfpj = nc.dram_tensor("bfpj", [128, L * PQ * JP], bf16, kind="ExternalInput")
    d_e2pf = nc.dram_tensor("e2pf", [128, NB], f32, kind="ExternalInput")
    d_wbig = nc.dram_tensor("wbig", [L, 128, 3 * 2048], fp8, kind="ExternalInput")
    # per layer: qk k-tiles (4*19) then fq k-tiles (4*19)
    d_qkfq = nc.dram_tensor("qkfq", [128, L * 8 * J], bf16, kind="ExternalInput")
    d_sfall = nc.dram_tensor("sfall", [JP, L * D], bf16, kind="ExternalInput")
    d_out = nc.dram_tensor("out", [NB, P], f32, kind="ExternalOutput")

    def bcast_mid(ap2d, n):
        # (p, k) AP -> (p, n, k) with step-0 broadcast in the middle
        return bass.AP(
            tensor=ap2d.tensor, offset=ap2d.offset,
            ap=[ap2d.ap[0], [0, n], ap2d.ap[1]],
        )

    def bcast4(ap2d):
        # (p, 32) AP -> (p, 4, 32) free (M=128) with step-0 group replication
        return bass.AP(
            tensor=ap2d.tensor, offset=ap2d.offset,
            ap=[ap2d.ap[0], [0, GG], ap2d.ap[1]],
        )

    with tile.TileContext(nc) as tc:
        with (
            tc.tile_pool(name="consts", bufs=1) as consts,
            tc.tile_pool(name="wpool", bufs=8) as wpool,
            tc.tile_pool(name="ap", bufs=3) as apool,
            tc.tile_pool(name="attn", bufs=1) as atp,
            tc.tile_pool(name="pm", bufs=4, space="PSUM") as pm,
            tc.tile_pool(name="pt", bufs=2, space="PSUM") as pt,
        ):
            # ---- constants ----
            cpk16 = consts.tile([128, C16_END], bf16)
            nc.sync.dma_start(out=cpk16, in_=d_cpk16[:, :])
            bfpj_all = consts.tile([128, L * PQ * JP], bf16)
            # per-layer DMAs so layer 0 isn't gated on the whole tensor
            for l in range(L):
                nc.sync.dma_start(
                    out=bfpj_all[:, l * PQ * JP : (l + 1) * PQ * JP],
                    in_=d_bfpj[:, l * PQ * JP : (l + 1) * PQ * JP],
                )
            e2pf = consts.tile([128, NB], f32)
            nc.sync.dma_start(out=e2pf, in_=d_e2pf[:, :])
            qkfq = consts.tile([128, L * 8 * J], bf16)
            nc.sync.dma_start(out=qkfq, in_=d_qkfq[:, :])
            sfall = consts.tile([JP, L * D], bf16)
            nc.sync.dma_start(out=sfall, in_=d_sfall[:, :])

            bfjp4 = cpk16[:NP, C_BFJP:C_E2P].rearrange("p (a b) -> p a b", b=PQP)
            e2p = cpk16[:128, C_E2P:C_E2T]        # (128, 32) bf16 group-major
            e2tp = cpk16[:NB, C_E2T : C_E2T + 128]  # (32, 128) bf16
            wz = cpk16[:, C_WZ:C_WHZ]             # (128, 4*81)
            whz = cpk16[:, C_WHZ:C_LGB]           # (128, 4*81)
            lgb = cpk16[:1, C_LGB : C_LGB + P]    # (1, 81)
            sel4 = cpk16[:4, C_SEL : C_SEL + NP]  # (4, 128) one-hot o selector
            b4 = cpk16[:4, C_B4:C16_END]          # (4, L*256): u bias | y bias

            ones_bf = consts.tile([1, NB], bf16)
            nc.vector.memset(ones_bf, 1.0)
            # persistent attention buffers (serial chain -> single-buffered)
            g3 = consts.tile([NP, JP], bf16)
            nc.vector.memset(g3, 1.0)       # pad col 19 stays 1.0 (gconst lane)
            e4 = consts.tile([NP, PQP], bf16)
            nc.vector.memset(e4, 0.0)       # pad col 21 stays 0
            sT_buf = atp.tile([JP, NB], bf16, tag="sTb")
            g_sb = atp.tile([NB, J], bf16, tag="gsb")
            s4 = atp.tile([NP, JR], bf16, tag="s4")
            rs = atp.tile([NP, 1], f32, tag="rs")
            t1 = atp.tile([NP, PQ * JP], bf16, tag="t1")
            t2 = atp.tile([NP, JR * PQP], bf16, tag="t2")
            dots = atp.tile([NP, PQ], f32, tag="dots")
            s4n = atp.tile([NP, JR], bf16, tag="s4n")
            recip = atp.tile([NB, 1], bf16, tag="recip")

            # residual stream: x^T as (128, 4*32), pure bf16
            xT_b = apool.tile([128, 4 * NB], bf16, tag="xb")
            nc.vector.memset(xT_b, 0.0)
            xT_mid = xT_b

            def xsl(t, kt):
                return t[:, kt * NB : (kt + 1) * NB]

            t1_3 = t1[:, :].rearrange("p (a b) -> p a b", b=JP)
            t2_3 = t2[:, :].rearrange("p (a b) -> p a b", b=PQP)

            prev = None  # (wb, uT, y_ps, sign) of layer l-1 pending fc1
            for l in range(L):
                wb = wpool.tile([128, 3 * 2048], fp8, tag="wb")
                nc.sync.dma_start(out=wb, in_=d_wbig[l, :, :])

                def wtile(mat, o, kt, wbx=None):
                    wbx = wb if wbx is None else wbx
                    c = ((mat * 4 + o) * 4 + kt) * 128
                    return wbx[:, c : c + 128]

                qk_l = qkfq[:, l * 8 * J : l * 8 * J + 4 * J]
                fq_l = qkfq[:, l * 8 * J + 4 * J : (l + 1) * 8 * J]
                bfpj_l = bfpj_all[
                    :, l * PQ * JP : (l + 1) * PQ * JP
                ].rearrange("p (a b) -> p a b", b=JP)
                aab = abs(alpha[l])

                # ---- g (32,19) then replicate to 128 group-major rows ----
                if l > 0:
                    wb_p, uT_p, y_p, sgn_p = prev
                    g_ps = pt.tile([NB, J], f32, tag="sp")
                    for kt in range(4):
                        nc.tensor.matmul(
                            g_ps, xsl(xT_mid, kt),
                            qk_l[:, kt * J : (kt + 1) * J],
                            start=(kt == 0), stop=False,
                        )
                    for kt in range(4):
                        nc.tensor.matmul(
                            g_ps, xsl(uT_p, kt),
                            fq_l[:, kt * J : (kt + 1) * J],
                            start=False, stop=(kt == 3),
                        )
                    nc.vector.tensor_copy(g_sb, g_ps)
                    g3_ps = pt.tile([NP, J], f32, tag="sp")
                    nc.tensor.matmul(g3_ps, e2tp, g_sb, start=True, stop=True)
                    nc.scalar.activation(g3[:, 0:J], g3_ps, Act.Copy)

                # ---- deferred fc1 of layer l-1 (hides under attention) ----
                if l > 0:
                    for o in range(4):
                        for kt in range(4):
                            nc.tensor.matmul(
                                xsl(y_p, o), wtile(2, o, kt, wb_p),
                                xsl(uT_p, kt),
                                start=False, stop=(kt == 3),
                            )

                # psum tiles + K=4 bias matmuls (no deps; run in PE slack)
                ft = pm.tile([128, 4 * NB], f32, tag="mm")
                u_ps = pm.tile([128, 4 * NB], f32, tag="mm")
                nc.tensor.matmul(
                    u_ps[:, :], b4[:, l * 2 * NP : l * 2 * NP + NP], sel4,
                    start=True, stop=False,
                )
                if l < L - 1:
                    y_ps = pm.tile([128, 4 * NB], f32, tag="mm")
                    nc.tensor.matmul(
                        y_ps[:, :], b4[:, l * 2 * NP + NP : (l + 1) * 2 * NP],
                        sel4, start=True, stop=False,
                    )

                # ---- dots = bfeat . g -> (128, 21)  (skipped at l=0) ----
                if l > 0:
                    nc.vector.tensor_tensor(
                        t1_3, bfpj_l, bcast_mid(g3[:, :], PQ), op=AluOp.mult
                    )
                    nc.vector.tensor_reduce(
                        dots, t1_3, axis=mybir.AxisListType.X, op=AluOp.add
                    )
                    # fc1 residual of layer l-1 (DVE slot after dots, before t2)
                    nxb = apool.tile([128, 4 * NB], bf16, tag="xb")
                    nc.vector.tensor_tensor(
                        nxb, xT_mid, y_p,
                        op=AluOp.add if sgn_p >= 0 else AluOp.subtract,
                    )
                    xT_b = nxb
                    # fin + fc0 x-parts now that x_l is final
                    for o in range(4):
                        for kt in range(4):
                            nc.tensor.matmul(
                                xsl(ft, o), wtile(0, o, kt), xsl(xT_b, kt),
                                start=(kt == 0), stop=False,
                            )
                    for o in range(4):
                        for kt in range(4):
                            nc.tensor.matmul(
                                xsl(u_ps, o), wtile(1, o, kt), xsl(xT_b, kt),
                                start=False, stop=False,
                            )
                with nc.allow_low_precision(reason="softmax rowsum"):
                    if l == 0:
                        # strided view of bfpj col 19: the precomputed dots
                        dots0 = bass.AP(
                            tensor=bfpj_l.tensor, offset=bfpj_l.offset + J,
                            ap=[bfpj_l.ap[0], bfpj_l.ap[1]],
                        )
                        nc.scalar.activation(
                            e4[:, 0:PQ], dots0, Act.Exp, accum_out=rs
                        )
                    else:
                        nc.scalar.activation(
                            e4[:, 0:PQ], dots, Act.Exp, accum_out=rs
                        )
                rsb_ps = pt.tile([NB, 1], f32, tag="sp")
                nc.tensor.matmul(rsb_ps, e2pf, rs, start=True, stop=True)

                # ---- s4 = sum_p e[b,p] bfjp[b,j,p]; row 19 = rowsum ----
                nc.vector.tensor_tensor(
                    t2_3, bfjp4, bcast_mid(e4[:, :], JR), op=AluOp.mult
                )
                with nc.allow_low_precision(reason="softmax recip in bf16"):
                    nc.vector.reciprocal(recip, rsb_ps)
                r3_ps = pt.tile([128, 1], f32, tag="sp")
                nc.tensor.matmul(r3_ps, e2tp, recip, start=True, stop=True)
                with nc.allow_low_precision(reason="attention s in bf16"):
                    nc.vector.tensor_reduce(
                        s4, t2_3, axis=mybir.AxisListType.X, op=AluOp.add
                    )
                nc.vector.tensor_scalar_mul(s4n, s4, r3_ps[:, :])
                # group-sum to s^T directly: (20,32) = s4n.T @ e2p; row19 = 1.0
                sT_ps = pt.tile([JR, NB], f32, tag="sp")
                nc.tensor.matmul(sT_ps, s4n, e2p, start=True, stop=True)
                nc.vector.tensor_copy(sT_buf[:, :], sT_ps)

                # ---- fin tail: += s @ [sfin;cfin] (K=20, bias inside) ----
                for o in range(4):
                    nc.tensor.matmul(
                        xsl(ft, o),
                        sfall[:, l * D + o * 128 : l * D + (o + 1) * 128],
                        sT_buf,
                        start=(l == 0), stop=True,
                    )
                # tv = alpha*relu(ft); fc0 tv-part rides on it so the
                # x_mid residual add leaves the critical chain
                tv = apool.tile([128, 4 * NB], bf16, tag="tv")
                nc.vector.tensor_scalar(
                    tv, ft, 0.0, float(alpha[l]), op0=AluOp.max, op1=AluOp.mult
                )
                for o in range(4):
                    for kt in range(4):
                        nc.tensor.matmul(
                            xsl(u_ps, o), wtile(1, o, kt), xsl(tv, kt),
                            start=False, stop=(kt == 3),
                        )
                nmid = apool.tile([128, 4 * NB], bf16, tag="xb")
                nc.vector.tensor_tensor(nmid, xT_b, tv, op=AluOp.add)
                xT_mid = nmid
                nuT = apool.tile([128, 4 * NB], bf16, tag="uT")
                nc.scalar.activation(nuT, u_ps, Act.Relu, scale=aab)
                uT = nuT
                # fc1 matmuls deferred to next layer (after its g matmuls)
                if l < L - 1:
                    prev = (wb, uT, y_ps, 1.0 if alpha[l] >= 0 else -1.0)

            # ---- head: lg = x_mid7 @ wz + u~7 @ whz + lgbias; log-softmax ----
            lg_ps = pt.tile([NB, P], f32, tag="sp")
            for kt in range(4):
                nc.tensor.matmul(
                    lg_ps, xsl(xT_mid, kt), wz[:, kt * P : (kt + 1) * P],
                    start=(kt == 0), stop=False,
                )
            nc.tensor.matmul(lg_ps, ones_bf, lgb, start=False, stop=False)
            for kt in range(4):
                nc.tensor.matmul(
                    lg_ps, xsl(uT, kt), whz[:, kt * P : (kt + 1) * P],
                    start=False, stop=(kt == 3),
                )
            ex = apool.tile([NB, P], bf16, tag="ex")
            se = apool.tile([NB, 1], f32, tag="se")
            with nc.allow_low_precision(reason="softmax exp scratch"):
                nc.scalar.activation(ex, lg_ps, Act.Exp, accum_out=se)
            lse = apool.tile([NB, 1], f32, tag="lse")
            nc.scalar.activation(lse, se, Act.Ln)
            outf = apool.tile([NB, P], f32, tag="outf")
            nc.vector.tensor_scalar(
                outf, lg_ps[:, :], lse[:, :], None, op0=AluOp.subtract
            )
            nc.sync.dma_start(out=d_out[:, :], in_=outf)

    nc.finalize()
    return nc


def kernel(**inputs):
    inp = {k: np.asarray(v, dtype=np.float32) for k, v in inputs.items()}
    pos = _positions()
    bfeat = _prepare(inp["obs"], pos)  # (256, 81, 19)
    qk_w, qk_b, afin, sfin, cfin = _fold(inp)
    alpha = inp["alpha"].astype(np.float32)

    wbig = np.concatenate(
        [_ktile_lhsT(afin), _ktile_lhsT(inp["fc0_w"]), _ktile_lhsT(inp["fc1_w"])],
        axis=2,
    ).astype(fp8_np)  # (L, 128, 6144)

    # g pipelining folds: fq_l = sign(a_{l-1}) * fc1_{l-1} @ qk_l
    fq = np.zeros((L, D, J), np.float32)
    gconst = qk_b.copy()
    for l in range(1, L):
        sgn = 1.0 if alpha[l - 1] >= 0 else -1.0
        fq[l] = sgn * (inp["fc1_w"][l - 1] @ qk_w[l])
        gconst[l] = qk_b[l] + alpha[l - 1] * (inp["fc1_b"][l - 1] @ qk_w[l])
    qkfq = np.zeros((128, L * 8 * J), np.float32)
    for l in range(L):
        qkfq[:, l * 8 * J : l * 8 * J + 4 * J] = (
            qk_w[l].reshape(4, 128, J).transpose(1, 0, 2).reshape(128, 4 * J)
        )
        qkfq[:, l * 8 * J + 4 * J : (l + 1) * 8 * J] = (
            fq[l].reshape(4, 128, J).transpose(1, 0, 2).reshape(128, 4 * J)
        )

    sfin_aug = np.concatenate([sfin, cfin[:, None, :]], axis=1)  # (L, 20, 512)
    sfall = np.ascontiguousarray(
        sfin_aug.transpose(1, 0, 2)
    ).reshape(JP, L * D).astype(bf16_np)

    # head folds
    wz = inp["head_w"] @ pos.reshape(P, POSD).T            # (512, 81)
    sgn7 = 1.0 if alpha[7] >= 0 else -1.0
    whz = sgn7 * (inp["fc1_w"][7] @ wz)                     # (512, 81)
    lgbias = alpha[7] * (inp["fc1_b"][7] @ wz)              # (81,)

    # group-major constants: partition p = gg*32 + b, cell = gg*21 + pq
    e2 = np.zeros((NP, NB), np.float32)
    for gg in range(GG):
        for b in range(NB):
            e2[gg * NB + b, b] = 1.0
    sel4 = np.zeros((4, NP), np.float32)
    for o in range(4):
        sel4[o, o * NB : (o + 1) * NB] = 1.0

    # shared cpk16 tail (same for all cores)
    cpk_tail = np.zeros((128, C16_END - C_E2P), np.float32)

    def tl(c0, c1):
        return cpk_tail[:, c0 - C_E2P : c1 - C_E2P]

    tl(C_E2P, C_E2T)[:] = e2
    tl(C_E2T, C_E2T + 128)[:NB] = e2.T
    tl(C_WZ, C_WHZ)[:] = wz.reshape(4, 128, P).transpose(1, 0, 2).reshape(128, 4 * P)
    tl(C_WHZ, C_LGB)[:] = whz.reshape(4, 128, P).transpose(1, 0, 2).reshape(128, 4 * P)
    tl(C_LGB, C_SEL)[0, :] = lgbias
    tl(C_SEL, C_B4)[:4] = sel4
    b4 = np.zeros((4, L * 2 * NP), np.float32)
    for l in range(L):
        b4[:, l * 2 * NP : l * 2 * NP + NP] = inp["fc0_b"][l].reshape(4, 128)
        if l < L - 1:
            sgn = 1.0 if alpha[l] >= 0 else -1.0
            b4[:, l * 2 * NP + NP : (l + 1) * 2 * NP] = (
                sgn * alpha[l] * inp["fc1_b"][l]
            ).reshape(4, 128)
    tl(C_B4, C16_END)[:4] = b4

    in_maps = []
    for cc in range(NCORES):
        bf = bfeat[cc * NB : (cc + 1) * NB]          # (32, 81, 19)
        # pad cells 81..83 with zeros, cell c -> (gg=c//21, pq=c%21)
        bfp = np.zeros((NB, GG * PQ, J), np.float32)
        bfp[:, :P, :] = bf
        bfg = bfp.reshape(NB, GG, PQ, J).transpose(1, 0, 2, 3)  # (gg,b,pq,j)
        # bfpj per layer: cols l*420 + pq*20 + j; col 19 = bf.gconst[l] or -40
        bfpj = np.zeros((128, L, PQ, JP), np.float32)
        dots_c = np.einsum("gbpj,lj->lgbp", bfg, gconst)        # (L,gg,b,pq)
        for l in range(L):
            bfpj[:, l, :, :J] = bfg.reshape(NP, PQ, J)
            bfpj[:, l, :, J] = dots_c[l].reshape(NP, PQ)
            for c in range(P, GG * PQ):
                bfpj[(c // PQ) * NB : (c // PQ + 1) * NB, l, c % PQ, J] = NEGBIG
        # bfjp: (p, j-rows 20, pq 22); row 19 = ones (softmax rowsum lane)
        bfjp = np.zeros((GG, NB, JR, PQP), np.float32)
        bfjp[:, :, :J, :PQ] = bfg.transpose(0, 1, 3, 2)
        bfjp[:, :, J, :PQ] = 1.0
        # pad cells must not contribute to the rowsum row
        for c in range(P, GG * PQ):
            bfjp[c // PQ, :, J, c % PQ] = 0.0
        cpk = np.zeros((128, C16_END), np.float32)
        cpk[:, C_BFJP:C_E2P] = bfjp.reshape(NP, JR * PQP)
        cpk[:, C_E2P:] = cpk_tail
        in_maps.append({
            "cpk16": cpk.astype(bf16_np),
            "bfpj": bfpj.reshape(128, L * PQ * JP).astype(bf16_np),
            "e2pf": e2,
            "wbig": wbig, "qkfq": qkfq.astype(bf16_np), "sfall": sfall,
        })

    nc = _build_nc([float(a) for a in alpha])
    res = run_bass_kernel_spmd(nc, in_maps, core_ids=list(range(NCORES)))
    out = np.concatenate([r["out"] for r in res.results], axis=0)  # (256, 81)
    return out.astype(np.float32)


# revision 13
# speedup vs baseline: 1.0359x; 1.0359x over previous
# Trainium2 Bass kernel for nn_AttnModel_64098091926054.
#
# Strategy: pure data parallel over batch (256 boards -> 32 per core x 8 cores).
# Host-side constant folding (softmax shift-invariance kills the x-dependent
# k-term; q_w folds into qk_w (512x19); kvx_v/Wv fold through fin_w).
#
# v5: chain-latency cuts on top of v4:
#  - gconst folded into a per-layer bfpj pad column (bf.gconst precomputed on
#    host) -> no fp32 gconst matmuls; layer 0 skips t1/dots entirely (exp
#    reads the precomputed column with a strided AP).
#  - g3 (128,19) computed directly with step-0-broadcast stationary APs
#    (x columns replicated 4x group-major) -> no g_sb copy, no replicate
#    matmul, no e2tp LDW on the chain.
#  - bfjp carries a ones j-row (JR=20): the t2/s4 reduce emits the softmax
#    row-sum row for free, s4/s4n have even free size (DVE 2x eligible),
#    and sT row 19 lands at exactly 1.0 = the cfin ones row.
#  - fc0/fc1 biases enter PSUM via ONE K=4 matmul each (bias rows x one-hot
#    selector) instead of 4 rank-1s.
#  - head: head_w@posT and fc1_7@(head_w@posT) folded on host; layer-7 fc1
#    never runs on PE; log-softmax skips the max shift (safe in f32); exp
#    reads lg straight from PSUM.

import numpy as np
import ml_dtypes

import concourse.bass as bass
import concourse.bacc as bacc
import concourse.mybir as mybir
import concourse.tile as tile
from concourse.bass_utils import run_bass_kernel_spmd

BS, D, L, B, P, POSD, J = 9, 512, 8, 256, 81, 12, 19
NCORES = 8
NB = B // NCORES          # 32 boards per core
GG, PQ = 4, 21            # 84 = 4 groups x 21 cells (3 pads)
NP = 128                  # partitions: p = gg*32 + b  (group-major)
JP = J + 1                # 20: j padded (pad col = gconst/-40 lane in bfpj)
JR = J + 1                # 20: bfjp j rows (19 features + ones row for rowsum)
PQP = PQ + 1              # 22: pq padded even
NEGBIG = -40.0
OFFSETS = [(-1, 0), (-1, 1), (0, -1), (0, 0), (0, 1), (-1, -1), (-1, 0)]

f32 = mybir.dt.float32
bf16 = mybir.dt.bfloat16
fp8 = mybir.dt.float8e4
bf16_np = ml_dtypes.bfloat16
fp8_np = ml_dtypes.float8_e4m3

# cpkA columns (bf16, 128 rows): e2p | e2tp | wz | whz | lgbias
A_E2P = 0
A_E2T = A_E2P + NB                 # 32
A_WZ = A_E2T + NP                  # 160
A_WHZ = A_WZ + 4 * P               # 484
A_LGB = A_WHZ + 4 * P              # 808
A_END = A_LGB + P                  # 889
# b4s columns (bf16, 4 rows): per-layer fc0/fc1 biases | one-hot selector
B_B4 = 0
B_SEL = B_B4 + L * 2 * NP          # 2048
B_END = B_SEL + NP                 # 2176
AluOp = mybir.AluOpType
Act = mybir.ActivationFunctionType


def _positions():
    lin = np.linspace(0.0, 1.0, BS, dtype=np.float32)
    rs, cs = np.meshgrid(lin, lin, indexing="ij")
    zs = (rs + cs) / 2.0
    xs = np.stack([rs, cs, zs], -1).astype(np.float32)
    feats = []
    for p in [4.0 / (BS - 1), 16.0 / (BS - 1)]:
        a = (2.0 * np.pi * xs / p).astype(np.float32)
        feats.append(np.concatenate([np.cos(a), np.sin(a)], -1).astype(np.float32))
    return np.concatenate(feats, -1)  # (9, 9, 12)


def _prepare(obs, pos):
    single = obs[..., 0] - obs[..., 1]
    aug = np.pad(single, ((0, 0), (1, 1), (1, 1)))
    w = aug.shape[-1]
    outs = [aug[:, 1 + r : w - 1 + r, 1 + c : w - 1 + c] for (r, c) in OFFSETS]
    neigh = np.stack(outs, -1)
    n = obs.shape[0]
    stack = np.concatenate(
        [neigh, np.broadcast_to(pos, (n,) + pos.shape)], -1
    ).astype(np.float32)
    return stack.reshape(n, P, J)  # (B, 81, 19)


def _fold(inp):
    """Host-side constant folding of weights. All f32 numpy, unscaled."""
    scale = np.float32(1.0 / np.sqrt(D))
    Wk = inp["kvb_w"][:, :, :D]                                   # (L,19,512)
    Wv = inp["kvb_w"][:, :, D:]
    kvx_v = inp["kvx_w"][:, :, D:]                                # (L,512,512)
    qk_w = np.einsum("ldh,ljh->ldj", inp["q_w"], Wk) * scale      # (L,512,19)
    qk_b = np.einsum("lh,ljh->lj", inp["q_b"], Wk) * scale        # (L,19)
    afin = np.einsum("lde,leh->ldh", kvx_v, inp["fin_w"])         # (L,512,512)
    sfin = np.einsum("lje,leh->ljh", Wv, inp["fin_w"])            # (L,19,512)
    bias_v = inp["kvx_b"][:, D:] + inp["kvb_b"][:, D:]
    cfin = np.einsum("le,leh->lh", bias_v, inp["fin_w"]) + inp["fin_b"]
    return qk_w, qk_b, afin, sfin, cfin


def _ktile_lhsT(W):
    """(L,512,512) -> (L,128,2048) with col ((o*4+kt)*128+m) = W[l,kt*128+k,o*128+m]."""
    Lx = W.shape[0]
    return np.ascontiguousarray(
        W.reshape(Lx, 4, 128, 4, 128).transpose(0, 2, 3, 1, 4).reshape(Lx, 128, 2048)
    )


def _build_nc(alpha):
    nc = bacc.Bacc("TRN2", target_bir_lowering=False, debug=False)

    d_cpkA = nc.dram_tensor("cpkA", [128, A_END], bf16, kind="ExternalInput")
    d_b4s = nc.dram_tensor("b4s", [4, B_END], bf16, kind="ExternalInput")
    d_bfjp = nc.dram_tensor("bfjp", [128, JR * PQP], bf16, kind="ExternalInput")
    d_bfpj = nc.dram_tensor("bfpj", [128, L * PQ * JP], bf16, kind="ExternalInput")
    d_e2pf = nc.dram_tensor("e2pf", [128, NB], f32, kind="ExternalInput")
    d_wbig = nc.dram_tensor("wbig", [L, 128, 3 * 2048], fp8, kind="ExternalInput")
    # per layer: qk k-tiles (4*19) then fq k-tiles (4*19)
    d_qkfq = nc.dram_tensor("qkfq", [128, L * 8 * J], bf16, kind="ExternalInput")
    d_sfall = nc.dram_tensor("sfall", [JP, L * D], bf16, kind="ExternalInput")
    d_out = nc.dram_tensor("out", [NB, P], f32, kind="ExternalOutput")

    def bcast_mid(ap2d, n):
        # (p, k) AP -> (p, n, k) with step-0 broadcast in the middle
        return bass.AP(
            tensor=ap2d.tensor, offset=ap2d.offset,
            ap=[ap2d.ap[0], [0, n], ap2d.ap[1]],
        )

    def bcast4(ap2d):
        # (p, 32) AP -> (p, 4, 32) free (M=128) with step-0 group replication
        return bass.AP(
            tensor=ap2d.tensor, offset=ap2d.offset,
            ap=[ap2d.ap[0], [0, GG], ap2d.ap[1]],
        )

    with tile.TileContext(nc) as tc:
        with (
            tc.tile_pool(name="consts", bufs=1) as consts,
            tc.tile_pool(name="wpool", bufs=8) as wpool,
            tc.tile_pool(name="ap", bufs=3) as apool,
            tc.tile_pool(name="attn", bufs=1) as atp,
            tc.tile_pool(name="pm", bufs=4, space="PSUM") as pm,
            tc.tile_pool(name="pt", bufs=2, space="PSUM") as pt,
        ):
            # ---- activation-table warm (EXP) before anything on Scalar ----
            warm = consts.tile([1, 2], f32)
            c00 = nc.const_aps.tensor(0.0, (1, 1), f32)
            nc.scalar.activation(warm[:, 0:1], c00, Act.Exp)

            # ---- constants; DMA issue order = layer-0 criticality ----
            # sync queue: small layer-0-critical loads
            cpkA = consts.tile([128, A_END], bf16)
            b4s = consts.tile([4, B_END], bf16)
            bfjp_t = consts.tile([128, JR * PQP], bf16)
            bfpj_all = consts.tile([128, L * PQ * JP], bf16)
            e2pf = consts.tile([128, NB], f32)
            qkfq = consts.tile([128, L * 8 * J], bf16)
            sfall = consts.tile([JP, L * D], bf16)

            nc.sync.dma_start(out=e2pf, in_=d_e2pf[:, :])
            nc.sync.dma_start(
                out=bfpj_all[:, : PQ * JP], in_=d_bfpj[:, : PQ * JP]
            )
            nc.sync.dma_start(out=bfjp_t, in_=d_bfjp[:, :])
            nc.sync.dma_start(out=b4s, in_=d_b4s[:, :])
            nc.sync.dma_start(out=cpkA, in_=d_cpkA[:, :])
            # bulk loads ride the gpsimd queue (issued inside the l=0 body,
            # after wbig[0], so they don't delay it)

            bfjp4 = bfjp_t[:NP, :].rearrange("p (a b) -> p a b", b=PQP)
            e2p = cpkA[:128, A_E2P:A_E2T]         # (128, 32) bf16 group-major
            e2tp = cpkA[:NB, A_E2T:A_WZ]          # (32, 128) bf16
            wz = cpkA[:, A_WZ:A_WHZ]              # (128, 4*81)
            whz = cpkA[:, A_WHZ:A_LGB]            # (128, 4*81)
            lgb = cpkA[:1, A_LGB:A_END]           # (1, 81)
            sel4 = b4s[:4, B_SEL:B_END]           # (4, 128) one-hot o selector
            b4 = b4s[:4, B_B4:B_SEL]              # (4, L*256): u bias | y bias

            ones_bf = consts.tile([1, NB], bf16)
            nc.vector.memset(ones_bf, 1.0)
            # persistent attention buffers (serial chain -> single-buffered)
            g3 = consts.tile([NP, JP], bf16)
            nc.vector.memset(g3, 1.0)       # pad col 19 stays 1.0 (gconst lane)
            e4 = consts.tile([NP, PQP], bf16)
            nc.vector.memset(e4, 0.0)       # pad col 21 stays 0
            sT_buf = atp.tile([JP, NB], bf16, tag="sTb")
            g_sb = atp.tile([NB, J], bf16, tag="gsb")
            s4 = atp.tile([NP, JR], bf16, tag="s4")
            rs = atp.tile([NP, 1], f32, tag="rs")
            t1 = atp.tile([NP, PQ * JP], bf16, tag="t1")
            t2 = atp.tile([NP, JR * PQP], bf16, tag="t2")
            dots = atp.tile([NP, PQ], f32, tag="dots")
            s4n = atp.tile([NP, JR], bf16, tag="s4n")
            recip = atp.tile([NB, 1], bf16, tag="recip")

            # residual stream: x^T as (128, 4*32), pure bf16
            xT_b = apool.tile([128, 4 * NB], bf16, tag="xb")
            nc.vector.memset(xT_b, 0.0)
            xT_mid = xT_b

            def xsl(t, kt):
                return t[:, kt * NB : (kt + 1) * NB]

            t1_3 = t1[:, :].rearrange("p (a b) -> p a b", b=JP)
            t2_3 = t2[:, :].rearrange("p (a b) -> p a b", b=PQP)

            prev = None  # (wb, uT, y_ps, sign) of layer l-1 pending fc1
            for l in range(L):
                wb = wpool.tile([128, 3 * 2048], fp8, tag="wb")
                nc.gpsimd.dma_start(out=wb, in_=d_wbig[l, :, :])
                if l == 0:
                    # bulk constants: issue behind wbig[0] on the gpsimd queue
                    nc.gpsimd.dma_start(out=sfall, in_=d_sfall[:, :])
                    nc.gpsimd.dma_start(out=qkfq, in_=d_qkfq[:, :])
                    nc.gpsimd.dma_start(
                        out=bfpj_all[:, PQ * JP :], in_=d_bfpj[:, PQ * JP :]
                    )

                def wtile(mat, o, kt, wbx=None):
                    wbx = wb if wbx is None else wbx
                    c = ((mat * 4 + o) * 4 + kt) * 128
                    return wbx[:, c : c + 128]

                qk_l = qkfq[:, l * 8 * J : l * 8 * J + 4 * J]
                fq_l = qkfq[:, l * 8 * J + 4 * J : (l + 1) * 8 * J]
                bfpj_l = bfpj_all[
                    :, l * PQ * JP : (l + 1) * PQ * JP
                ].rearrange("p (a b) -> p a b", b=JP)
                aab = abs(alpha[l])

                # ---- g (32,19) then replicate to 128 group-major rows ----
                if l > 0:
                    wb_p, uT_p, y_p, sgn_p = prev
                    g_ps = pt.tile([NB, J], f32, tag="sp")
                    for kt in range(4):
                        nc.tensor.matmul(
                            g_ps, xsl(xT_mid, kt),
                            qk_l[:, kt * J : (kt + 1) * J],
                            start=(kt == 0), stop=False,
                        )
                    for kt in range(4):
                        nc.tensor.matmul(
                            g_ps, xsl(uT_p, kt),
                            fq_l[:, kt * J : (kt + 1) * J],
                            start=False, stop=(kt == 3),
                        )
                    nc.vector.tensor_copy(g_sb, g_ps)
                    g3_ps = pt.tile([NP, J], f32, tag="sp")
                    nc.tensor.matmul(g3_ps, e2tp, g_sb, start=True, stop=True)
                    nc.scalar.activation(g3[:, 0:J], g3_ps, Act.Copy)

                # ---- deferred fc1 of layer l-1 (hides under attention) ----
                if l > 0:
                    for o in range(4):
                        for kt in range(4):
                            nc.tensor.matmul(
                                xsl(y_p, o), wtile(2, o, kt, wb_p),
                                xsl(uT_p, kt),
                                start=False, stop=(kt == 3),
                            )

                # psum tiles + K=4 bias matmuls (no deps; run in PE slack)
                ft = pm.tile([128, 4 * NB], f32, tag="mm")
                u_ps = pm.tile([128, 4 * NB], f32, tag="mm")
                nc.tensor.matmul(
                    u_ps[:, :], b4[:, l * 2 * NP : l * 2 * NP + NP], sel4,
                    start=True, stop=False,
                )
                if l < L - 1:
                    y_ps = pm.tile([128, 4 * NB], f32, tag="mm")
                    nc.tensor.matmul(
                        y_ps[:, :], b4[:, l * 2 * NP + NP : (l + 1) * 2 * NP],
                        sel4, start=True, stop=False,
                    )

                # ---- dots = bfeat . g -> (128, 21)  (skipped at l=0) ----
                if l > 0:
                    nc.vector.tensor_tensor(
                        t1_3, bfpj_l, bcast_mid(g3[:, :], PQ), op=AluOp.mult
                    )
                    nc.vector.tensor_reduce(
                        dots, t1_3, axis=mybir.AxisListType.X, op=AluOp.add
                    )
                    # fc1 residual of layer l-1 (DVE slot after dots, before t2)
                    nxb = apool.tile([128, 4 * NB], bf16, tag="xb")
                    nc.vector.tensor_tensor(
                        nxb, xT_mid, y_p,
                        op=AluOp.add if sgn_p >= 0 else AluOp.subtract,
                    )
                    xT_b = nxb
                    # fin + fc0 x-parts now that x_l is final
                    for o in range(4):
                        for kt in range(4):
                            nc.tensor.matmul(
                                xsl(ft, o), wtile(0, o, kt), xsl(xT_b, kt),
                                start=(kt == 0), stop=False,
                            )
                    for o in range(4):
                        for kt in range(4):
                            nc.tensor.matmul(
                                xsl(u_ps, o), wtile(1, o, kt), xsl(xT_b, kt),
                                start=False, stop=False,
                            )
                with nc.allow_low_precision(reason="softmax rowsum"):
                    if l == 0:
                        # strided view of bfpj col 19: the precomputed dots
                        dots0 = bass.AP(
                            tensor=bfpj_l.tensor, offset=bfpj_l.offset + J,
                            ap=[bfpj_l.ap[0], bfpj_l.ap[1]],
                        )
                        nc.scalar.activation(
                            e4[:, 0:PQ], dots0, Act.Exp, accum_out=rs
                        )
                    else:
                        nc.scalar.activation(
                            e4[:, 0:PQ], dots, Act.Exp, accum_out=rs
                        )
                rsb_ps = pt.tile([NB, 1], f32, tag="sp")
                nc.tensor.matmul(rsb_ps, e2pf, rs, start=True, stop=True)

                # ---- s4 = sum_p e[b,p] bfjp[b,j,p]; row 19 = rowsum ----
                nc.vector.tensor_tensor(
                    t2_3, bfjp4, bcast_mid(e4[:, :], JR), op=AluOp.mult
                )
                with nc.allow_low_precision(reason="softmax recip in bf16"):
                    nc.vector.reciprocal(recip, rsb_ps)
                r3_ps = pt.tile([128, 1], f32, tag="sp")
                nc.tensor.matmul(r3_ps, e2tp, recip, start=True, stop=True)
                with nc.allow_low_precision(reason="attention s in bf16"):
                    nc.vector.tensor_reduce(
                        s4, t2_3, axis=mybir.AxisListType.X, op=AluOp.add
                    )
                nc.vector.tensor_scalar_mul(s4n, s4, r3_ps[:, :])
                # group-sum to s^T directly: (20,32) = s4n.T @ e2p; row19 = 1.0
                sT_ps = pt.tile([JR, NB], f32, tag="sp")
                nc.tensor.matmul(sT_ps, s4n, e2p, start=True, stop=True)
                nc.vector.tensor_copy(sT_buf[:, :], sT_ps)

                # ---- fin tail: += s @ [sfin;cfin] (K=20, bias inside) ----
                for o in range(4):
                    nc.tensor.matmul(
                        xsl(ft, o),
                        sfall[:, l * D + o * 128 : l * D + (o + 1) * 128],
                        sT_buf,
                        start=(l == 0), stop=True,
                    )
                # tv = alpha*relu(ft); fc0 tv-part rides on it so the
                # x_mid residual add leaves the critical chain
                tv = apool.tile([128, 4 * NB], bf16, tag="tv")
                nc.vector.tensor_scalar(
                    tv, ft, 0.0, float(alpha[l]), op0=AluOp.max, op1=AluOp.mult
                )
                for o in range(4):
                    for kt in range(4):
                        nc.tensor.matmul(
                            xsl(u_ps, o), wtile(1, o, kt), xsl(tv, kt),
                            start=False, stop=(kt == 3),
                        )
                nmid = apool.tile([128, 4 * NB], bf16, tag="xb")
                nc.vector.tensor_tensor(nmid, xT_b, tv, op=AluOp.add)
                xT_mid = nmid
                nuT = apool.tile([128, 4 * NB], bf16, tag="uT")
                nc.scalar.activation(nuT, u_ps, Act.Relu, scale=aab)
                uT = nuT
                if l == 0:
                    # warm the LN activation table in layer-0 Scalar slack
                    nc.scalar.activation(warm[:, 1:2], c00, Act.Ln)
                # fc1 matmuls deferred to next layer (after its g matmuls)
                if l < L - 1:
                    prev = (wb, uT, y_ps, 1.0 if alpha[l] >= 0 else -1.0)

            # ---- head: lg = x_mid7 @ wz + u~7 @ whz + lgbias; log-softmax ----
            lg_ps = pt.tile([NB, P], f32, tag="sp")
            for kt in range(4):
                nc.tensor.matmul(
                    lg_ps, xsl(xT_mid, kt), wz[:, kt * P : (kt + 1) * P],
                    start=(kt == 0), stop=False,
                )
            nc.tensor.matmul(lg_ps, ones_bf, lgb, start=False, stop=False)
            for kt in range(4):
                nc.tensor.matmul(
                    lg_ps, xsl(uT, kt), whz[:, kt * P : (kt + 1) * P],
                    start=False, stop=(kt == 3),
                )
            ex = apool.tile([NB, P], bf16, tag="ex")
            se = apool.tile([NB, 1], f32, tag="se")
            with nc.allow_low_precision(reason="softmax exp scratch"):
                nc.scalar.activation(ex, lg_ps, Act.Exp, accum_out=se)
            lse = apool.tile([NB, 1], f32, tag="lse")
            nc.scalar.activation(lse, se, Act.Ln)
            outf = apool.tile([NB, P], f32, tag="outf")
            nc.vector.tensor_scalar(
                outf, lg_ps[:, :], lse[:, :], None, op0=AluOp.subtract
            )
            nc.sync.dma_start(out=d_out[:, :], in_=outf)

    nc.finalize()
    return nc


def kernel(**inputs):
    inp = {k: np.asarray(v, dtype=np.float32) for k, v in inputs.items()}
    pos = _positions()
    bfeat = _prepare(inp["obs"], pos)  # (256, 81, 19)
    qk_w, qk_b, afin, sfin, cfin = _fold(inp)
    alpha = inp["alpha"].astype(np.float32)

    wbig = np.concatenate(
        [_ktile_lhsT(afin), _ktile_lhsT(inp["fc0_w"]), _ktile_lhsT(inp["fc1_w"])],
        axis=2,
    ).astype(fp8_np)  # (L, 128, 6144)

    # g pipelining folds: fq_l = sign(a_{l-1}) * fc1_{l-1} @ qk_l
    fq = np.zeros((L, D, J), np.float32)
    gconst = qk_b.copy()
    for l in range(1, L):
        sgn = 1.0 if alpha[l - 1] >= 0 else -1.0
        fq[l] = sgn * (inp["fc1_w"][l - 1] @ qk_w[l])
        gconst[l] = qk_b[l] + alpha[l - 1] * (inp["fc1_b"][l - 1] @ qk_w[l])
    qkfq = np.zeros((128, L * 8 * J), np.float32)
    for l in range(L):
        qkfq[:, l * 8 * J : l * 8 * J + 4 * J] = (
            qk_w[l].reshape(4, 128, J).transpose(1, 0, 2).reshape(128, 4 * J)
        )
        qkfq[:, l * 8 * J + 4 * J : (l + 1) * 8 * J] = (
            fq[l].reshape(4, 128, J).transpose(1, 0, 2).reshape(128, 4 * J)
        )

    sfin_aug = np.concatenate([sfin, cfin[:, None, :]], axis=1)  # (L, 20, 512)
    sfall = np.ascontiguousarray(
        sfin_aug.transpose(1, 0, 2)
    ).reshape(JP, L * D).astype(bf16_np)

    # head folds
    wz = inp["head_w"] @ pos.reshape(P, POSD).T            # (512, 81)
    sgn7 = 1.0 if alpha[7] >= 0 else -1.0
    whz = sgn7 * (inp["fc1_w"][7] @ wz)                     # (512, 81)
    lgbias = alpha[7] * (inp["fc1_b"][7] @ wz)              # (81,)

    # group-major constants: partition p = gg*32 + b, cell = gg*21 + pq
    e2 = np.zeros((NP, NB), np.float32)
    for gg in range(GG):
        for b in range(NB):
            e2[gg * NB + b, b] = 1.0
    sel4 = np.zeros((4, NP), np.float32)
    for o in range(4):
        sel4[o, o * NB : (o + 1) * NB] = 1.0

    # shared constant blocks (same for all cores)
    cpkA = np.zeros((128, A_END), np.float32)
    cpkA[:, A_E2P:A_E2T] = e2
    cpkA[:NB, A_E2T:A_WZ] = e2.T
    cpkA[:, A_WZ:A_WHZ] = wz.reshape(4, 128, P).transpose(1, 0, 2).reshape(128, 4 * P)
    cpkA[:, A_WHZ:A_LGB] = whz.reshape(4, 128, P).transpose(1, 0, 2).reshape(128, 4 * P)
    cpkA[0, A_LGB:A_END] = lgbias
    b4s = np.zeros((4, B_END), np.float32)
    b4s[:, B_SEL:B_END] = sel4
    for l in range(L):
        b4s[:, B_B4 + l * 2 * NP : B_B4 + l * 2 * NP + NP] = (
            inp["fc0_b"][l].reshape(4, 128)
        )
        if l < L - 1:
            sgn = 1.0 if alpha[l] >= 0 else -1.0
            b4s[:, B_B4 + l * 2 * NP + NP : B_B4 + (l + 1) * 2 * NP] = (
                sgn * alpha[l] * inp["fc1_b"][l]
            ).reshape(4, 128)
    cpkA16 = cpkA.astype(bf16_np)
    b4s16 = b4s.astype(bf16_np)

    in_maps = []
    for cc in range(NCORES):
        bf = bfeat[cc * NB : (cc + 1) * NB]          # (32, 81, 19)
        # pad cells 81..83 with zeros, cell c -> (gg=c//21, pq=c%21)
        bfp = np.zeros((NB, GG * PQ, J), np.float32)
        bfp[:, :P, :] = bf
        bfg = bfp.reshape(NB, GG, PQ, J).transpose(1, 0, 2, 3)  # (gg,b,pq,j)
        # bfpj per layer: cols l*420 + pq*20 + j; col 19 = bf.gconst[l] or -40
        bfpj = np.zeros((128, L, PQ, JP), np.float32)
        dots_c = np.einsum("gbpj,lj->lgbp", bfg, gconst)        # (L,gg,b,pq)
        for l in range(L):
            bfpj[:, l, :, :J] = bfg.reshape(NP, PQ, J)
            bfpj[:, l, :, J] = dots_c[l].reshape(NP, PQ)
            for c in range(P, GG * PQ):
                bfpj[(c // PQ) * NB : (c // PQ + 1) * NB, l, c % PQ, J] = NEGBIG
        # bfjp: (p, j-rows 20, pq 22); row 19 = ones (softmax rowsum lane)
        bfjp = np.zeros((GG, NB, JR, PQP), np.float32)
        bfjp[:, :, :J, :PQ] = bfg.transpose(0, 1, 3, 2)
        bfjp[:, :, J, :PQ] = 1.0
        # pad cells must not contribute to the rowsum row
        for c in range(P, GG * PQ):
            bfjp[c // PQ, :, J, c % PQ] = 0.0
        in_maps.append({
            "cpkA": cpkA16, "b4s": b4s16,
            "bfjp": bfjp.reshape(NP, JR * PQP).astype(bf16_np),
            "bfpj": bfpj.reshape(128, L * PQ * JP).astype(bf16_np),
            "e2pf": e2,
            "wbig": wbig, "qkfq": qkfq.astype(bf16_np), "sfall": sfall,
        })

    nc = _build_nc([float(a) for a in alpha])
    res = run_bass_kernel_spmd(nc, in_maps, core_ids=list(range(NCORES)))
    out = np.concatenate([r["out"] for r in res.results], axis=0)  # (256, 81)
    return out.astype(np.float32)
